# revision 21
# baseline (speedup 1.0000x reference)
"""Trainium2 Bass kernel for nn_CrossAttentionBlock (B=8, N=1024, C=768, H=12).

Sharding: data-parallel over the batch dim — each of the 8 NeuronCores runs the
full cross-attention block for one batch element. No collectives.

The final output is LN(query + attn_out) where ||attn_out|| is ~3% of
||query||: the attention path tolerates large relative error, so every matmul
runs in fp8 (e4m3) with the DoubleRow perf mode (2 contraction subtiles per
pass, 0.5 cycles/row on the PE).  The residual + LayerNorm path stays fp32.

Per-core dataflow:
  1. Host marshals: activations/weights transposed to feature-major fp8.
     Wq/Wk rows (output features) are PERMUTED so the projection writes Q^T/K^T
     directly in a [32-partition x (head, half) plane] layout that the scores
     matmul can consume with DoubleRow (contraction d=64 split as 2x32).
     query+bo is pre-folded fp32 for the residual.
  2. Projections on PE (fp8 DR, contraction 768 = 3 pair-passes). Q/K evacs:
     first slabs on ScalarE (idle then), rest on DVE. V is token-major with a
     ones column per head (V_aug) so attn@V also produces softmax row sums.
  3. Attention per head h: S^T[k,q] via DR (lhsT = K plane [32,2,128]);
     exp fused into the PSUM->SBUF evac on ScalarE (bounded scores, no max
     subtraction) writing fp8 E-pair tiles [128,2,1024]; O_aug[65,q] += V_aug
     pairs @ E pairs (DR, 4 pair passes). Row 64 = softmax denominator:
     DVE reciprocal -> DRAM bounce -> stride-0 broadcast DMA to [64,q] ->
     one DVE multiply evacuates normalized AO (fp8, feature-major).
  4. Out-proj on PE (fp8 DR) + epilogue per 128-token tile: residual add
     (query+bo preloaded), LayerNorm via bn_stats/bn_aggr + Sqrt(ACT) +
     reciprocal + fused (x-mu)*rs on DVE, gamma on GpSimd, beta on DVE.
"""

import json

import ml_dtypes
import numpy as np

import concourse.bass as bass
import concourse.mybir as mybir
import concourse.tile as tile

B, N, C, H, D = 8, 1024, 768, 12, 64
KB = C // 128  # feature-dim 128-blocks
TB = N // 128  # token-dim 128-blocks
SCALE = D ** -0.5
EPS = 1e-5
F32 = mybir.dt.float32
BF16 = mybir.dt.bfloat16
FP8 = mybir.dt.float8e4
AF = mybir.ActivationFunctionType
ALU = mybir.AluOpType
DR = mybir.MatmulPerfMode.DoubleRow
FP8_NP = ml_dtypes.float8_e4m3

# Output-feature permutation for Q/K: PE matmul operands must start at
# partition 0/32/64, so only 3 of 4 32-partition blocks per slab are usable.
# Pad the projection to 8 output slabs: slab s, partition p holds original
# feature f = h*64 + i*32 + (p%32) with h = (s//2)*3 + p//32, i = s%2; the
# p//32 == 3 block is dummy (zero weight columns, never read).  The scores
# matmul reads head h as partitions (h%3)*32..+32, slabs (h//3)*2..+2 — a
# [32, 2, *] DoubleRow operand at a legal base partition.
KB8 = 8  # padded Q/K slab count
_PERM = np.full(KB8 * 128, -1, dtype=np.int64)
for _s in range(KB8):
    for _p in range(128):
        if _p // 32 < 3:
            _h = (_s // 2) * 3 + _p // 32
            _PERM[_s * 128 + _p] = _h * 64 + (_s % 2) * 32 + (_p % 32)

# ---------------------------------------------------------------------------
# Workaround: this walrus build rejects instructions with more than one
# semaphore wait ("Too many sync wait commands").  Legalize the BIR by hoisting
# excess waits onto same-engine NoOps inserted right before the instruction.
# ---------------------------------------------------------------------------
_MAX_WAITS = 1
_legal_counter = [0]


def _legalize_waits(bir_json: bytes) -> bytes:
    m = json.loads(bir_json)
    changed = False
    for fn in m.get("functions", []):
        for bb in fn.get("blocks", []):
            out = []
            for inst in bb.get("instructions", []):
                si = inst.get("sync_info") or {}
                waits = si.get("on_wait") or []
                if len(waits) > _MAX_WAITS:
                    changed = True
                    extra = waits[_MAX_WAITS:]
                    si["on_wait"] = waits[:_MAX_WAITS]
                    for i in range(0, len(extra), _MAX_WAITS):
                        _legal_counter[0] += 1
                        nop = {
                            "engine": inst["engine"],
                            "ins": [],
                            "name": f"I-legalw-{_legal_counter[0]}",
                            "opcode": "NoOp",
                            "outs": [],
                            "sync_info": {
                                "on_update": [],
                                "on_wait": extra[i : i + _MAX_WAITS],
                            },
                        }
                        if "debug" in inst:
                            nop["debug"] = inst["debug"]
                        out.append(nop)
                out.append(inst)
            bb["instructions"] = out
    return json.dumps(m).encode() if changed else bir_json


_hooked = False


def _install_compile_hook():
    global _hooked
    if _hooked:
        return
    _hooked = True
    import concourse.bass_utils as bu

    orig = bu.compile_bir_kernel

    def compile_bir_kernel(bir_json, tmpdir, neff_name="file.neff"):
        return orig(_legalize_waits(bir_json), tmpdir, neff_name)

    bu.compile_bir_kernel = compile_bir_kernel
    try:
        import concourse.bass2jax as b2j

        b2j.compile_bir_kernel = compile_bir_kernel
    except ImportError:
        pass


# ---------------------------------------------------------------------------
# Kernel builder
# ---------------------------------------------------------------------------

def _dram_ap(t, offset, ap):
    return bass.AP(t, offset, ap)


def build_nc(identity_ln: bool = False) -> bass.Bass:
    nc = bass.Bass()

    qT_d = nc.dram_tensor("qT", [C, N], FP8, kind="ExternalInput")
    cT_d = nc.dram_tensor("cT", [C, N], FP8, kind="ExternalInput")
    Wq_d = nc.dram_tensor("WqTp", [C, KB8 * 128], FP8, kind="ExternalInput")
    Wk_d = nc.dram_tensor("WkTp", [C, KB8 * 128], FP8, kind="ExternalInput")
    Wv_d = nc.dram_tensor("WvT", [C, C], FP8, kind="ExternalInput")
    Wo_d = nc.dram_tensor("WoT", [C, C], FP8, kind="ExternalInput")
    bqp = nc.dram_tensor("bqp", [KB8 * 128], F32, kind="ExternalInput")
    bkp = nc.dram_tensor("bkp", [KB8 * 128], F32, kind="ExternalInput")
    bv = nc.dram_tensor("bv", [C], F32, kind="ExternalInput")
    qbo = nc.dram_tensor("qbo", [N, C], F32, kind="ExternalInput")
    gamma = nc.dram_tensor("ln_gamma", [C], F32, kind="ExternalInput")
    beta = nc.dram_tensor("ln_beta", [C], F32, kind="ExternalInput")
    r_dram = nc.dram_tensor("r_scratch", [H * N], F32, kind="Internal")
    out_t = nc.dram_tensor("out", [N, C], F32, kind="ExternalOutput")

    with tile.TileContext(nc) as tc:
        _body(tc, nc, (qT_d, cT_d), (Wq_d, Wk_d, Wv_d, Wo_d),
              (bqp, bkp, bv), qbo, gamma, beta, r_dram, out_t, identity_ln)
    return nc


def _proj_slab(nc, psA, WT, srcT, s, dst_cb):
    """One 128-feature output slab of a projection, in two 512-column chunks
    on a [128, 512] double-buffered PSUM ring (PE overlaps the evacuation of
    chunk n with the matmuls of chunk n+1)."""
    for ch in range(2):
        pj = psA.tile([128, 512], F32, name="pj", tag="pj", bufs=2)
        for kp in range(KB // 2):
            nc.tensor.matmul(
                pj,
                WT[:, 2 * kp : 2 * kp + 2, s * 128 : (s + 1) * 128],
                srcT[:, 2 * kp : 2 * kp + 2, ch * 512 : (ch + 1) * 512],
                start=(kp == 0), stop=(kp == KB // 2 - 1),
                perf_mode=DR,
            )
        dst_cb(pj, ch)


def _body(tc, nc, actTs, WTs, bs, qbo, gamma, beta, r_dram, out_t, identity_ln):
    qT_d, cT_d = actTs
    Wq_d, Wk_d, Wv_d, Wo_d = WTs
    bqp, bkp, bv = bs

    with (
        tc.tile_pool(name="singles", bufs=1) as singles,
        tc.tile_pool(name="feat", bufs=1) as feat,
    ):
        # ---- tiny bias DMAs first (they gate the Q/K evacuations) -------
        bq_sb = singles.tile([128, KB8], F32, name="bq_sb")
        nc.sync.dma_start(out=bq_sb, in_=_dram_ap(bqp, 0, [[1, 128], [128, KB8]]))
        bk_sb = singles.tile([128, KB8], F32, name="bk_sb")
        nc.sync.dma_start(out=bk_sb, in_=_dram_ap(bkp, 0, [[1, 128], [128, KB8]]))

        # ---- long-lived tensors (critical DMAs first: scores path) ------
        qT = feat.tile([128, KB, N], FP8, name="qT")
        nc.sync.dma_start(
            out=qT, in_=_dram_ap(qT_d, 0, [[N, 128], [128 * N, KB], [1, N]])
        )
        WqT = feat.tile([128, KB, KB8 * 128], FP8, name="WqT")
        WkT = feat.tile([128, KB, KB8 * 128], FP8, name="WkT")
        # first two output slabs arrive first: they gate head group 0
        for wT, w_d, c0, c1 in ((WqT, Wq_d, 0, 256),):
            nc.sync.dma_start(
                out=wT[:, :, c0:c1],
                in_=_dram_ap(
                    w_d, c0,
                    [[KB8 * 128, 128], [128 * KB8 * 128, KB], [1, c1 - c0]],
                ),
            )
        cT = feat.tile([128, KB, N], FP8, name="cT")
        nc.sync.dma_start(
            out=cT, in_=_dram_ap(cT_d, 0, [[N, 128], [128 * N, KB], [1, N]])
        )
        nc.sync.dma_start(
            out=WkT[:, :, 0:256],
            in_=_dram_ap(
                Wk_d, 0,
                [[KB8 * 128, 128], [128 * KB8 * 128, KB], [1, 256]],
            ),
        )
        for wT, w_d in ((WqT, Wq_d), (WkT, Wk_d)):
            nc.sync.dma_start(
                out=wT[:, :, 256 : KB8 * 128],
                in_=_dram_ap(
                    w_d, 256,
                    [[KB8 * 128, 128], [128 * KB8 * 128, KB],
                     [1, KB8 * 128 - 256]],
                ),
            )
        WvT = feat.tile([128, KB, C], FP8, name="WvT")
        WoT = feat.tile([128, KB, C], FP8, name="WoT")
        for wT, w_d in ((WvT, Wv_d), (WoT, Wo_d)):
            nc.sync.dma_start(
                out=wT, in_=_dram_ap(w_d, 0, [[C, 128], [128 * C, KB], [1, C]])
            )

        # ---- constants (after the critical path) ------------------------
        bv_bc = singles.tile([128, C], F32, name="bv_bc")
        nc.sync.dma_start(out=bv_bc, in_=_dram_ap(bv, 0, [[0, 128], [1, C]]))
        gamma_bc = singles.tile([128, C], F32, name="gamma_bc")
        nc.sync.dma_start(out=gamma_bc, in_=_dram_ap(gamma, 0, [[0, 128], [1, C]]))
        beta_bc = singles.tile([128, C], F32, name="beta_bc")
        nc.sync.dma_start(out=beta_bc, in_=_dram_ap(beta, 0, [[0, 128], [1, C]]))
        eps_t = singles.tile([128, 1], F32, name="eps_t")
        nc.vector.memset(eps_t, EPS)
        ones64 = singles.tile([1, D], BF16, name="ones64")
        nc.vector.memset(ones64, 1.0)
        qbo_sb = feat.tile([128, TB, C], F32, name="qbo_sb")
        nc.sync.dma_start(
            out=qbo_sb, in_=_dram_ap(qbo, 0, [[C, 128], [128 * C, TB], [1, C]])
        )

        # Q^T/K^T live in 4 per-head-group tiles so a head's scores only
        # depend on its own group's evacuations (precise WAR tracking).
        QTg = [feat.tile([128, 2, N], FP8, name=f"QTg{g}") for g in range(4)]
        KTg = [feat.tile([128, 2, N], FP8, name=f"KTg{g}") for g in range(4)]
        V_sb = feat.tile([128, TB, H, 128], FP8, name="V_sb")
        AO = feat.tile([128, KB, N], FP8, name="AO")

        nc.gpsimd.memset(V_sb[:, :, :, D + 1 : 128], 0.0)
        nc.gpsimd.memset(V_sb[:, :, :, D : D + 1], 1.0)

        with (
            tc.tile_pool(name="psA", bufs=1, space="PSUM") as psA,
            tc.tile_pool(name="psS", bufs=1, space="PSUM") as psS,
            tc.tile_pool(name="psO", bufs=1, space="PSUM") as psO,
            tc.tile_pool(name="attn", bufs=1) as attn,
        ):
            def qk_evac_act(dstG, s, b_sb):
                def cb(pj, ch):
                    nc.scalar.activation(
                        out=dstG[s // 2][:, s % 2, ch * 512 : (ch + 1) * 512],
                        in_=pj, func=AF.Identity,
                        bias=b_sb[:, s : s + 1], scale=1.0,
                    )
                return cb

            def qk_evac_dve(dstG, s, b_sb):
                def cb(pj, ch):
                    nc.vector.tensor_scalar(
                        out=dstG[s // 2][:, s % 2, ch * 512 : (ch + 1) * 512],
                        in0=pj,
                        scalar1=b_sb[:, s : s + 1], scalar2=None,
                        op0=ALU.add,
                    )
                return cb

            def v_proj(tb):
                for ci, (c0, c1) in enumerate(((0, 512), (512, C))):
                    nh = (c1 - c0) // D
                    h0 = c0 // D
                    pv = psA.tile([128, 512], F32, name="pv", tag="pj", bufs=2)
                    for kp in range(KB // 2):
                        nc.tensor.matmul(
                            pv[:, 0 : c1 - c0],
                            cT[:, 2 * kp : 2 * kp + 2, tb * 128 : (tb + 1) * 128],
                            WvT[:, 2 * kp : 2 * kp + 2, c0:c1],
                            start=(kp == 0), stop=(kp == KB // 2 - 1),
                            perf_mode=DR,
                        )
                    nc.vector.tensor_add(
                        out=V_sb[:, tb, h0 : h0 + nh, 0:D],
                        in0=pv[:, 0 : c1 - c0].rearrange("p (h d) -> p h d", h=nh),
                        in1=bv_bc[:, c0:c1].rearrange("p (h d) -> p h d", h=nh),
                    )

            carry = {}  # h -> Epair tile whose half 0 is already exp'd

            def emit_scores(h, kt):
                pb = (h % 3) * 32
                g = h // 3
                S = psS.tile([128, N], F32, name="S", tag="S", bufs=2)
                lhsT = KTg[g][pb : pb + 32, :, kt * 128 : (kt + 1) * 128]
                for ch in range(2):
                    nc.tensor.matmul(
                        S[:, ch * 512 : (ch + 1) * 512],
                        lhsT,
                        QTg[g][pb : pb + 32, :, ch * 512 : (ch + 1) * 512],
                        start=True, stop=True, perf_mode=DR,
                    )
                return S

            def emit_exp(Epair, half, S):
                nc.scalar.activation(
                    out=Epair[:, half, :], in_=S, func=AF.Exp, scale=SCALE,
                )

            def emit_av(O, h, pr, Epair, start, stop):
                for ch in range(2):
                    nc.tensor.matmul(
                        O[:, ch * 512 : (ch + 1) * 512],
                        V_sb[:, 2 * pr : 2 * pr + 2, h, :],  # [128, 2, 128]
                        Epair[:, :, ch * 512 : (ch + 1) * 512],
                        start=start, stop=stop,
                        perf_mode=DR,
                    )

            def head(h, prefetch_next=True):
                O = psO.tile([128, N], F32, name="O", tag="O", bufs=1)
                # ALL exps first, ALL attn@V accumulations clumped at the head
                # end: the O bank is then written only during the last ~2us of
                # the head, which gives the previous head's normalize chain
                # (recip -> DRAM bounce -> multiply, ~6us of mostly DMA
                # latency) the full head period to retire before the clump.
                pairs = []
                if h in carry:
                    pairs.append(carry.pop(h))
                else:
                    first = attn.tile([128, 2, N], FP8, name="E", tag="E",
                                      bufs=11)
                    emit_exp(first, 0, emit_scores(h, 0))
                    emit_exp(first, 1, emit_scores(h, 1))
                    pairs.append(first)
                last = h == H - 1
                for pr in range(1, TB // 2):
                    Epair = attn.tile([128, 2, N], FP8, name="E", tag="E",
                                      bufs=11)
                    for half in range(2):
                        kt = pr * 2 + half
                        emit_exp(Epair, half, emit_scores(h, kt))
                    pairs.append(Epair)
                    if last:
                        # tail head: accumulate eagerly so AO completes right
                        # after the final exp instead of in a serial clump
                        emit_av(O, h, pr - 1, pairs[pr - 1],
                                start=(pr == 1), stop=False)
                if prefetch_next and h + 1 < H:
                    # Pre-emit the next head's full first pair (scores + exps)
                    # so the ACT stream stays dense across the boundary.
                    nEpair = attn.tile([128, 2, N], FP8, name="E", tag="E",
                                       bufs=11)
                    emit_exp(nEpair, 0, emit_scores(h + 1, 0))
                    emit_exp(nEpair, 1, emit_scores(h + 1, 1))
                    carry[h + 1] = nEpair
                if last:
                    emit_av(O, h, TB // 2 - 1, pairs[TB // 2 - 1],
                            start=False, stop=True)
                else:
                    for pr in range(TB // 2):
                        emit_av(O, h, pr, pairs[pr], start=(pr == 0),
                                stop=(pr == TB // 2 - 1))
                # softmax denominator -> reciprocal -> partition broadcast
                if h < H - 2:
                    r = attn.tile([1, N], F32, name="r", tag="r", bufs=2)
                    nc.vector.reciprocal(out=r, in_=O[D : D + 1, :])
                    nc.sync.dma_start(
                        out=_dram_ap(r_dram, h * N, [[1, 1], [1, N]]), in_=r
                    )
                    bc = attn.tile([D, N], F32, name="bc", tag="bc", bufs=2)
                    nc.sync.dma_start(
                        out=bc, in_=_dram_ap(r_dram, h * N, [[0, D], [1, N]])
                    )
                    nc.vector.tensor_mul(
                        out=AO[(h % 2) * D : (h % 2) * D + D, h // 2, :],
                        in0=O[0:D, :], in1=bc,
                    )
                else:
                    # tail heads: skip the DRAM bounce (two DMA-latency hops)
                    # and broadcast on the PE through the now-idle projection
                    # PSUM ring, with ACT evacuations (ACT idles post-exp).
                    r_bf = attn.tile([1, N], BF16, name="r_bf", tag="r", bufs=2)
                    with nc.allow_low_precision(reason="bf16 bcast of recip"):
                        nc.vector.reciprocal(out=r_bf, in_=O[D : D + 1, :])
                    bcw = attn.tile([D, N], F32, name="bcw", tag="bcw",
                                    bufs=2)
                    for ch in range(2):
                        bc_ps = psA.tile([128, 512], F32, name="pj", tag="pj",
                                         bufs=2)
                        nc.tensor.matmul(
                            bc_ps[0:D, :],
                            ones64,
                            r_bf[:, ch * 512 : (ch + 1) * 512],
                            start=True, stop=True,
                        )
                        nc.scalar.activation(
                            out=bcw[:, ch * 512 : (ch + 1) * 512],
                            in_=bc_ps[0:D, :], func=AF.Identity,
                        )
                    nc.vector.tensor_mul(
                        out=AO[(h % 2) * D : (h % 2) * D + D, h // 2, :],
                        in0=O[0:D, :], in1=bcw,
                    )

            # ---- emission schedule --------------------------------------
            # Early slabs on ACT (idle before exps), late slabs on DVE.
            for s in (0, 1):
                _proj_slab(nc, psA, WqT, qT, s, qk_evac_act(QTg, s, bq_sb))
                _proj_slab(nc, psA, WkT, cT, s, qk_evac_act(KTg, s, bk_sb))
            for tb in range(TB):
                v_proj(tb)
            head(0)
            head(1)
            for s in (2, 3):
                _proj_slab(nc, psA, WqT, qT, s, qk_evac_dve(QTg, s, bq_sb))
                _proj_slab(nc, psA, WkT, cT, s, qk_evac_dve(KTg, s, bk_sb))
            head(2)
            head(3)
            for s in (4, 5):
                _proj_slab(nc, psA, WqT, qT, s, qk_evac_dve(QTg, s, bq_sb))
                _proj_slab(nc, psA, WkT, cT, s, qk_evac_dve(KTg, s, bk_sb))
            head(4)
            head(5)
            for s in (6, 7):
                _proj_slab(nc, psA, WqT, qT, s, qk_evac_dve(QTg, s, bq_sb))
                _proj_slab(nc, psA, WkT, cT, s, qk_evac_dve(KTg, s, bk_sb))
            for h in range(6, H):
                head(h)

        # ---- stage 4: out-proj + residual + LayerNorm -------------------
        with (
            tc.tile_pool(name="psY", bufs=1, space="PSUM") as psY,
            tc.tile_pool(name="epi", bufs=1) as epi,
        ):
            for tb in range(TB):
                Y = psY.tile([128, C], F32, name="Y", tag="Y", bufs=2)
                for fp in range(KB // 2):
                    lhsT = AO[:, 2 * fp : 2 * fp + 2, tb * 128 : (tb + 1) * 128]
                    for c0, c1 in ((0, 512), (512, C)):
                        nc.tensor.matmul(
                            Y[:, c0:c1], lhsT,
                            WoT[:, 2 * fp : 2 * fp + 2, c0:c1],
                            start=(fp == 0), stop=(fp == KB // 2 - 1),
                            perf_mode=DR,
                        )
                # residual add with running sum(x); then E[x^2] via a Square
                # activation with accumulator (ACT is idle during the tail)
                x1 = epi.tile([128, C], F32, name="x1", tag="x1", bufs=3)
                sx = epi.tile([128, 1], F32, name="sx", tag="sx", bufs=2)
                nc.vector.scalar_tensor_tensor(
                    out=x1, in0=Y, scalar=0.0, in1=qbo_sb[:, tb, :],
                    op0=ALU.add, op1=ALU.add, accum_out=sx,
                )
                scr = epi.tile([128, C], BF16, name="scr", tag="scr", bufs=2)
                ssq = epi.tile([128, 1], F32, name="ssq", tag="ssq", bufs=2)
                if identity_ln and tb % 2 == 0:
                    nc.vector.scalar_tensor_tensor(
                        out=scr, in0=x1, scalar=0.0, in1=x1,
                        op0=ALU.add, op1=ALU.mult, accum_out=ssq,
                    )
                else:
                    nc.scalar.activation(
                        out=scr, in_=x1, func=AF.Square, accum_out=ssq,
                    )
                mu = epi.tile([128, 1], F32, name="mu", tag="mu", bufs=2)
                nc.vector.tensor_scalar(
                    out=mu, in0=sx, scalar1=1.0 / C, scalar2=None, op0=ALU.mult,
                )
                musq = epi.tile([128, 1], F32, name="musq", tag="musq", bufs=2)
                nc.vector.tensor_scalar(
                    out=musq, in0=mu, scalar1=mu[:, 0:1], scalar2=None,
                    op0=ALU.mult,
                )
                var = epi.tile([128, 1], F32, name="var", tag="var", bufs=2)
                nc.vector.scalar_tensor_tensor(
                    out=var, in0=ssq, scalar=1.0 / C, in1=musq,
                    op0=ALU.mult, op1=ALU.subtract,
                )
                sd = epi.tile([128, 1], F32, name="sd", tag="sd", bufs=2)
                nc.scalar.activation(
                    out=sd, in_=var, func=AF.Sqrt,
                    bias=eps_t[:, 0:1], scale=1.0,
                )
                rs = epi.tile([128, 1], F32, name="rs", tag="rs", bufs=2)
                nc.vector.reciprocal(out=rs, in_=sd)
                nmr = epi.tile([128, 1], F32, name="nmr", tag="nmr", bufs=2)
                nc.vector.tensor_scalar(
                    out=nmr, in0=mu, scalar1=rs[:, 0:1], scalar2=-1.0,
                    op0=ALU.mult, op1=ALU.mult,
                )
                xn = epi.tile([128, C], F32, name="xn", tag="xn", bufs=3)
                if identity_ln and tb % 2 == 1:
                    nc.vector.tensor_scalar(
                        out=xn, in0=x1, scalar1=mu[:, 0:1],
                        scalar2=rs[:, 0:1],
                        op0=ALU.subtract, op1=ALU.mult,
                    )
                else:
                    nc.scalar.activation(
                        out=xn, in_=x1, func=AF.Identity,
                        scale=rs[:, 0:1], bias=nmr[:, 0:1],
                    )
                if identity_ln:
                    out_sb = xn
                else:
                    xg = epi.tile([128, C], F32, name="xg", tag="xg", bufs=3)
                    nc.gpsimd.tensor_mul(out=xg, in0=xn, in1=gamma_bc)
                    xb = epi.tile([128, C], F32, name="xb", tag="xb", bufs=3)
                    nc.vector.tensor_add(out=xb, in0=xg, in1=beta_bc)
                    out_sb = xb
                nc.sync.dma_start(
                    out=_dram_ap(out_t, tb * 128 * C, [[C, 128], [1, C]]),
                    in_=out_sb,
                )


# ---------------------------------------------------------------------------
# Entry point
# ---------------------------------------------------------------------------
_nc_cache = {}


def _get_nc(identity_ln: bool = False):
    if identity_ln not in _nc_cache:
        _install_compile_hook()
        _nc_cache[identity_ln] = build_nc(identity_ln)
    return _nc_cache[identity_ln]


def make_in_maps(inputs: dict) -> list:
    """Host-side marshaling: shard over batch, transpose to feature-major,
    cast matmul operands to fp8 (e4m3), permute Wq/Wk output features."""
    arrs = {k: np.asarray(v, dtype=np.float32) for k, v in inputs.items()}
    def _permw(W):
        Wp = np.zeros((KB8 * 128, C), dtype=np.float32)
        ok = _PERM >= 0
        Wp[ok] = W[_PERM[ok]]
        return np.ascontiguousarray(Wp.T.astype(FP8_NP))

    def _permb(b):
        bp = np.zeros(KB8 * 128, dtype=np.float32)
        ok = _PERM >= 0
        bp[ok] = b[_PERM[ok]]
        return bp

    shared = {
        "WqTp": _permw(arrs["Wq"]),
        "WkTp": _permw(arrs["Wk"]),
        "WvT": np.ascontiguousarray(arrs["Wv"].T.astype(FP8_NP)),
        "WoT": np.ascontiguousarray(arrs["Wo"].T.astype(FP8_NP)),
        "bqp": _permb(arrs["bq"]),
        "bkp": _permb(arrs["bk"]),
        "bv": arrs["bv"],
        "ln_gamma": arrs["ln_gamma"], "ln_beta": arrs["ln_beta"],
    }
    in_maps = []
    for b in range(B):
        m = dict(shared)
        m["qT"] = np.ascontiguousarray(arrs["query"][b].T.astype(FP8_NP))
        m["cT"] = np.ascontiguousarray(arrs["context"][b].T.astype(FP8_NP))
        m["qbo"] = np.ascontiguousarray(arrs["query"][b] + arrs["bo"])
        in_maps.append(m)
    return in_maps


def kernel(**inputs) -> np.ndarray:
    from concourse.bass_utils import run_bass_kernel_spmd

    g = np.asarray(inputs["ln_gamma"], dtype=np.float32)
    b = np.asarray(inputs["ln_beta"], dtype=np.float32)
    identity_ln = bool(np.all(g == 1.0) and np.all(b == 0.0))
    nc = _get_nc(identity_ln)
    in_maps = make_in_maps(inputs)
    res = run_bass_kernel_spmd(nc, in_maps, core_ids=list(range(B)))
    return np.stack([r["out"] for r in res.results]).astype(np.float32)


# revision 22
# speedup vs baseline: 1.0372x; 1.0372x over previous
"""Trainium2 Bass kernel for nn_CrossAttentionBlock (B=8, N=1024, C=768, H=12).

Sharding: data-parallel over the batch dim — each of the 8 NeuronCores runs the
full cross-attention block for one batch element. No collectives.

The final output is LN(query + attn_out) where ||attn_out|| is ~3% of
||query||: the attention path tolerates large relative error, so every matmul
runs in fp8 (e4m3) with the DoubleRow perf mode (2 contraction subtiles per
pass, 0.5 cycles/row on the PE).  The residual + LayerNorm path stays fp32.

Per-core dataflow:
  1. Host marshals: activations/weights transposed to feature-major fp8.
     Wq/Wk rows (output features) are PERMUTED so the projection writes Q^T/K^T
     directly in a [32-partition x (head, half) plane] layout that the scores
     matmul can consume with DoubleRow (contraction d=64 split as 2x32).
     query+bo is pre-folded fp32 for the residual.
  2. Projections on PE (fp8 DR, contraction 768 = 3 pair-passes). Q/K evacs:
     first slabs on ScalarE (idle then), rest on DVE. V is token-major with a
     ones column per head (V_aug) so attn@V also produces softmax row sums.
  3. Attention per head h: S^T[k,q] via DR (lhsT = K plane [32,2,128]);
     exp fused into the PSUM->SBUF evac on ScalarE (bounded scores, no max
     subtraction) writing fp8 E-pair tiles [128,2,1024]; O_aug[65,q] += V_aug
     pairs @ E pairs (DR, 4 pair passes). Row 64 = softmax denominator:
     DVE reciprocal -> DRAM bounce -> stride-0 broadcast DMA to [64,q] ->
     one DVE multiply evacuates normalized AO (fp8, feature-major).
  4. Out-proj on PE (fp8 DR) + epilogue per 128-token tile: residual add
     (query+bo preloaded), LayerNorm via bn_stats/bn_aggr + Sqrt(ACT) +
     reciprocal + fused (x-mu)*rs on DVE, gamma on GpSimd, beta on DVE.
"""

import json

import ml_dtypes
import numpy as np

import concourse.bass as bass
import concourse.mybir as mybir
import concourse.tile as tile

B, N, C, H, D = 8, 1024, 768, 12, 64
KB = C // 128  # feature-dim 128-blocks
TB = N // 128  # token-dim 128-blocks
SCALE = D ** -0.5
EPS = 1e-5
F32 = mybir.dt.float32
BF16 = mybir.dt.bfloat16
FP8 = mybir.dt.float8e4
AF = mybir.ActivationFunctionType
ALU = mybir.AluOpType
DR = mybir.MatmulPerfMode.DoubleRow
FP8_NP = ml_dtypes.float8_e4m3

# Output-feature permutation for Q/K: PE matmul operands must start at
# partition 0/32/64, so only 3 of 4 32-partition blocks per slab are usable.
# Pad the projection to 8 output slabs: slab s, partition p holds original
# feature f = h*64 + i*32 + (p%32) with h = (s//2)*3 + p//32, i = s%2; the
# p//32 == 3 block is dummy (zero weight columns, never read).  The scores
# matmul reads head h as partitions (h%3)*32..+32, slabs (h//3)*2..+2 — a
# [32, 2, *] DoubleRow operand at a legal base partition.
KB8 = 8  # padded Q/K slab count
_PERM = np.full(KB8 * 128, -1, dtype=np.int64)
for _s in range(KB8):
    for _p in range(128):
        if _p // 32 < 3:
            _h = (_s // 2) * 3 + _p // 32
            _PERM[_s * 128 + _p] = _h * 64 + (_s % 2) * 32 + (_p % 32)

# ---------------------------------------------------------------------------
# Workaround: this walrus build rejects instructions with more than one
# semaphore wait ("Too many sync wait commands").  Legalize the BIR by hoisting
# excess waits onto same-engine NoOps inserted right before the instruction.
# ---------------------------------------------------------------------------
_MAX_WAITS = 1
_legal_counter = [0]


def _legalize_waits(bir_json: bytes) -> bytes:
    m = json.loads(bir_json)
    changed = False
    for fn in m.get("functions", []):
        for bb in fn.get("blocks", []):
            out = []
            for inst in bb.get("instructions", []):
                si = inst.get("sync_info") or {}
                waits = si.get("on_wait") or []
                if len(waits) > _MAX_WAITS:
                    changed = True
                    extra = waits[_MAX_WAITS:]
                    si["on_wait"] = waits[:_MAX_WAITS]
                    for i in range(0, len(extra), _MAX_WAITS):
                        _legal_counter[0] += 1
                        nop = {
                            "engine": inst["engine"],
                            "ins": [],
                            "name": f"I-legalw-{_legal_counter[0]}",
                            "opcode": "NoOp",
                            "outs": [],
                            "sync_info": {
                                "on_update": [],
                                "on_wait": extra[i : i + _MAX_WAITS],
                            },
                        }
                        if "debug" in inst:
                            nop["debug"] = inst["debug"]
                        out.append(nop)
                out.append(inst)
            bb["instructions"] = out
    return json.dumps(m).encode() if changed else bir_json


_hooked = False


def _install_compile_hook():
    global _hooked
    if _hooked:
        return
    _hooked = True
    import concourse.bass_utils as bu

    orig = bu.compile_bir_kernel

    def compile_bir_kernel(bir_json, tmpdir, neff_name="file.neff"):
        return orig(_legalize_waits(bir_json), tmpdir, neff_name)

    bu.compile_bir_kernel = compile_bir_kernel
    try:
        import concourse.bass2jax as b2j

        b2j.compile_bir_kernel = compile_bir_kernel
    except ImportError:
        pass


# ---------------------------------------------------------------------------
# Kernel builder
# ---------------------------------------------------------------------------

def _dram_ap(t, offset, ap):
    return bass.AP(t, offset, ap)


def build_nc(identity_ln: bool = False) -> bass.Bass:
    nc = bass.Bass()

    qT_d = nc.dram_tensor("qT", [C, N], FP8, kind="ExternalInput")
    cT_d = nc.dram_tensor("cT", [C, N], FP8, kind="ExternalInput")
    Wq_d = nc.dram_tensor("WqTp", [C, KB8 * 128], FP8, kind="ExternalInput")
    Wk_d = nc.dram_tensor("WkTp", [C, KB8 * 128], FP8, kind="ExternalInput")
    Wv_d = nc.dram_tensor("WvT", [C, C], FP8, kind="ExternalInput")
    Wo_d = nc.dram_tensor("WoT", [C, C], FP8, kind="ExternalInput")
    bqp = nc.dram_tensor("bqp", [KB8 * 128], F32, kind="ExternalInput")
    bkp = nc.dram_tensor("bkp", [KB8 * 128], F32, kind="ExternalInput")
    bv = nc.dram_tensor("bv", [C], F32, kind="ExternalInput")
    qbo = nc.dram_tensor("qbo", [N, C], F32, kind="ExternalInput")
    gamma = nc.dram_tensor("ln_gamma", [C], F32, kind="ExternalInput")
    beta = nc.dram_tensor("ln_beta", [C], F32, kind="ExternalInput")
    r_dram = nc.dram_tensor("r_scratch", [H * N], F32, kind="Internal")
    out_t = nc.dram_tensor("out", [N, C], F32, kind="ExternalOutput")

    with tile.TileContext(nc) as tc:
        _body(tc, nc, (qT_d, cT_d), (Wq_d, Wk_d, Wv_d, Wo_d),
              (bqp, bkp, bv), qbo, gamma, beta, r_dram, out_t, identity_ln)
    return nc


def _proj_slab(nc, psA, WT, srcT, s, dst_cb):
    """One 128-feature output slab of a projection, in two 512-column chunks
    on a [128, 512] double-buffered PSUM ring (PE overlaps the evacuation of
    chunk n with the matmuls of chunk n+1)."""
    for ch in range(2):
        pj = psA.tile([128, 512], F32, name="pj", tag="pj", bufs=2)
        for kp in range(KB // 2):
            nc.tensor.matmul(
                pj,
                WT[:, 2 * kp : 2 * kp + 2, s * 128 : (s + 1) * 128],
                srcT[:, 2 * kp : 2 * kp + 2, ch * 512 : (ch + 1) * 512],
                start=(kp == 0), stop=(kp == KB // 2 - 1),
                perf_mode=DR,
            )
        dst_cb(pj, ch)


def _body(tc, nc, actTs, WTs, bs, qbo, gamma, beta, r_dram, out_t, identity_ln):
    qT_d, cT_d = actTs
    Wq_d, Wk_d, Wv_d, Wo_d = WTs
    bqp, bkp, bv = bs

    with (
        tc.tile_pool(name="singles", bufs=1) as singles,
        tc.tile_pool(name="feat", bufs=1) as feat,
    ):
        # ---- tiny bias DMAs first (they gate the Q/K evacuations) -------
        bq_sb = singles.tile([128, KB8], F32, name="bq_sb")
        nc.sync.dma_start(out=bq_sb, in_=_dram_ap(bqp, 0, [[1, 128], [128, KB8]]))
        bk_sb = singles.tile([128, KB8], F32, name="bk_sb")
        nc.sync.dma_start(out=bk_sb, in_=_dram_ap(bkp, 0, [[1, 128], [128, KB8]]))

        # ---- long-lived tensors (critical DMAs first: scores path) ------
        qT = feat.tile([128, KB, N], FP8, name="qT")
        nc.sync.dma_start(
            out=qT, in_=_dram_ap(qT_d, 0, [[N, 128], [128 * N, KB], [1, N]])
        )
        WqT = feat.tile([128, KB, KB8 * 128], FP8, name="WqT")
        WkT = feat.tile([128, KB, KB8 * 128], FP8, name="WkT")
        # first two output slabs arrive first: they gate head group 0
        for wT, w_d, c0, c1 in ((WqT, Wq_d, 0, 256),):
            nc.sync.dma_start(
                out=wT[:, :, c0:c1],
                in_=_dram_ap(
                    w_d, c0,
                    [[KB8 * 128, 128], [128 * KB8 * 128, KB], [1, c1 - c0]],
                ),
            )
        cT = feat.tile([128, KB, N], FP8, name="cT")
        nc.sync.dma_start(
            out=cT, in_=_dram_ap(cT_d, 0, [[N, 128], [128 * N, KB], [1, N]])
        )
        nc.sync.dma_start(
            out=WkT[:, :, 0:256],
            in_=_dram_ap(
                Wk_d, 0,
                [[KB8 * 128, 128], [128 * KB8 * 128, KB], [1, 256]],
            ),
        )
        for wT, w_d in ((WqT, Wq_d), (WkT, Wk_d)):
            nc.sync.dma_start(
                out=wT[:, :, 256 : KB8 * 128],
                in_=_dram_ap(
                    w_d, 256,
                    [[KB8 * 128, 128], [128 * KB8 * 128, KB],
                     [1, KB8 * 128 - 256]],
                ),
            )
        WvT = feat.tile([128, KB, C], FP8, name="WvT")
        WoT = feat.tile([128, KB, C], FP8, name="WoT")
        for wT, w_d in ((WvT, Wv_d), (WoT, Wo_d)):
            nc.sync.dma_start(
                out=wT, in_=_dram_ap(w_d, 0, [[C, 128], [128 * C, KB], [1, C]])
            )

        # ---- constants (after the critical path) ------------------------
        bv_bc = singles.tile([128, C], F32, name="bv_bc")
        nc.sync.dma_start(out=bv_bc, in_=_dram_ap(bv, 0, [[0, 128], [1, C]]))
        gamma_bc = singles.tile([128, C], F32, name="gamma_bc")
        nc.sync.dma_start(out=gamma_bc, in_=_dram_ap(gamma, 0, [[0, 128], [1, C]]))
        beta_bc = singles.tile([128, C], F32, name="beta_bc")
        nc.sync.dma_start(out=beta_bc, in_=_dram_ap(beta, 0, [[0, 128], [1, C]]))
        eps_t = singles.tile([128, 1], F32, name="eps_t")
        nc.vector.memset(eps_t, EPS)
        ones64 = singles.tile([1, D], BF16, name="ones64")
        nc.vector.memset(ones64, 1.0)
        qbo_sb = feat.tile([128, TB, C], F32, name="qbo_sb")
        nc.sync.dma_start(
            out=qbo_sb, in_=_dram_ap(qbo, 0, [[C, 128], [128 * C, TB], [1, C]])
        )

        # Q^T/K^T live in 4 per-head-group tiles so a head's scores only
        # depend on its own group's evacuations (precise WAR tracking).
        QTg = [feat.tile([128, 2, N], FP8, name=f"QTg{g}") for g in range(4)]
        KTg = [feat.tile([128, 2, N], FP8, name=f"KTg{g}") for g in range(4)]
        V_sb = feat.tile([128, TB, H, 128], FP8, name="V_sb")
        AO = feat.tile([128, KB, N], FP8, name="AO")

        nc.gpsimd.memset(V_sb[:, :, :, D + 1 : 128], 0.0)
        nc.gpsimd.memset(V_sb[:, :, :, D : D + 1], 1.0)

        with (
            tc.tile_pool(name="psA", bufs=1, space="PSUM") as psA,
            tc.tile_pool(name="psS", bufs=1, space="PSUM") as psS,
            tc.tile_pool(name="psO", bufs=1, space="PSUM") as psO,
            tc.tile_pool(name="attn", bufs=1) as attn,
        ):
            def qk_evac_act(dstG, s, b_sb):
                def cb(pj, ch):
                    nc.scalar.activation(
                        out=dstG[s // 2][:, s % 2, ch * 512 : (ch + 1) * 512],
                        in_=pj, func=AF.Identity,
                        bias=b_sb[:, s : s + 1], scale=1.0,
                    )
                return cb

            def qk_evac_dve(dstG, s, b_sb):
                def cb(pj, ch):
                    nc.vector.tensor_scalar(
                        out=dstG[s // 2][:, s % 2, ch * 512 : (ch + 1) * 512],
                        in0=pj,
                        scalar1=b_sb[:, s : s + 1], scalar2=None,
                        op0=ALU.add,
                    )
                return cb

            def v_proj(tb):
                for ci, (c0, c1) in enumerate(((0, 512), (512, C))):
                    nh = (c1 - c0) // D
                    h0 = c0 // D
                    pv = psA.tile([128, 512], F32, name="pv", tag="pj", bufs=2)
                    for kp in range(KB // 2):
                        nc.tensor.matmul(
                            pv[:, 0 : c1 - c0],
                            cT[:, 2 * kp : 2 * kp + 2, tb * 128 : (tb + 1) * 128],
                            WvT[:, 2 * kp : 2 * kp + 2, c0:c1],
                            start=(kp == 0), stop=(kp == KB // 2 - 1),
                            perf_mode=DR,
                        )
                    nc.vector.tensor_add(
                        out=V_sb[:, tb, h0 : h0 + nh, 0:D],
                        in0=pv[:, 0 : c1 - c0].rearrange("p (h d) -> p h d", h=nh),
                        in1=bv_bc[:, c0:c1].rearrange("p (h d) -> p h d", h=nh),
                    )

            carry = {}  # h -> Epair tile whose half 0 is already exp'd

            def emit_scores(h, kt):
                pb = (h % 3) * 32
                g = h // 3
                S = psS.tile([128, N], F32, name="S", tag="S", bufs=2)
                lhsT = KTg[g][pb : pb + 32, :, kt * 128 : (kt + 1) * 128]
                for ch in range(2):
                    nc.tensor.matmul(
                        S[:, ch * 512 : (ch + 1) * 512],
                        lhsT,
                        QTg[g][pb : pb + 32, :, ch * 512 : (ch + 1) * 512],
                        start=True, stop=True, perf_mode=DR,
                    )
                return S

            def emit_exp(Epair, half, S):
                nc.scalar.activation(
                    out=Epair[:, half, :], in_=S, func=AF.Exp, scale=SCALE,
                )

            def emit_av(O, h, pr, Epair, start, stop):
                for ch in range(2):
                    nc.tensor.matmul(
                        O[:, ch * 512 : (ch + 1) * 512],
                        V_sb[:, 2 * pr : 2 * pr + 2, h, :],  # [128, 2, 128]
                        Epair[:, :, ch * 512 : (ch + 1) * 512],
                        start=start, stop=stop,
                        perf_mode=DR,
                    )

            def head(h, prefetch_next=True):
                O = psO.tile([128, N], F32, name="O", tag="O", bufs=1)
                # ALL exps first, ALL attn@V accumulations clumped at the head
                # end: the O bank is then written only during the last ~2us of
                # the head, which gives the previous head's normalize chain
                # (recip -> DRAM bounce -> multiply, ~6us of mostly DMA
                # latency) the full head period to retire before the clump.
                pairs = []
                if h in carry:
                    pairs.append(carry.pop(h))
                else:
                    first = attn.tile([128, 2, N], FP8, name="E", tag="E",
                                      bufs=8)
                    emit_exp(first, 0, emit_scores(h, 0))
                    emit_exp(first, 1, emit_scores(h, 1))
                    pairs.append(first)
                last = h == H - 1
                for pr in range(1, TB // 2):
                    Epair = attn.tile([128, 2, N], FP8, name="E", tag="E",
                                      bufs=8)
                    for half in range(2):
                        kt = pr * 2 + half
                        emit_exp(Epair, half, emit_scores(h, kt))
                    pairs.append(Epair)
                    if last:
                        # tail head: accumulate eagerly so AO completes right
                        # after the final exp instead of in a serial clump
                        emit_av(O, h, pr - 1, pairs[pr - 1],
                                start=(pr == 1), stop=False)
                if prefetch_next and h + 1 < H:
                    # Pre-emit the next head's full first pair (scores + exps)
                    # so the ACT stream stays dense across the boundary.
                    nEpair = attn.tile([128, 2, N], FP8, name="E", tag="E",
                                       bufs=8)
                    emit_exp(nEpair, 0, emit_scores(h + 1, 0))
                    emit_exp(nEpair, 1, emit_scores(h + 1, 1))
                    carry[h + 1] = nEpair
                if last:
                    emit_av(O, h, TB // 2 - 1, pairs[TB // 2 - 1],
                            start=False, stop=True)
                else:
                    for pr in range(TB // 2):
                        emit_av(O, h, pr, pairs[pr], start=(pr == 0),
                                stop=(pr == TB // 2 - 1))
                # softmax denominator -> reciprocal -> partition broadcast
                if h < H - 2:
                    r = attn.tile([1, N], F32, name="r", tag="r", bufs=2)
                    nc.vector.reciprocal(out=r, in_=O[D : D + 1, :])
                    nc.sync.dma_start(
                        out=_dram_ap(r_dram, h * N, [[1, 1], [1, N]]), in_=r
                    )
                    bc = attn.tile([D, N], F32, name="bc", tag="bc", bufs=2)
                    nc.sync.dma_start(
                        out=bc, in_=_dram_ap(r_dram, h * N, [[0, D], [1, N]])
                    )
                    nc.vector.tensor_mul(
                        out=AO[(h % 2) * D : (h % 2) * D + D, h // 2, :],
                        in0=O[0:D, :], in1=bc,
                    )
                else:
                    # tail heads: skip the DRAM bounce (two DMA-latency hops)
                    # and broadcast on the PE through the now-idle projection
                    # PSUM ring, with ACT evacuations (ACT idles post-exp).
                    r_bf = attn.tile([1, N], BF16, name="r_bf", tag="r", bufs=2)
                    with nc.allow_low_precision(reason="bf16 bcast of recip"):
                        nc.vector.reciprocal(out=r_bf, in_=O[D : D + 1, :])
                    bcw = attn.tile([D, N], F32, name="bcw", tag="bcw",
                                    bufs=2)
                    for ch in range(2):
                        bc_ps = psA.tile([128, 512], F32, name="pj", tag="pj",
                                         bufs=2)
                        nc.tensor.matmul(
                            bc_ps[0:D, :],
                            ones64,
                            r_bf[:, ch * 512 : (ch + 1) * 512],
                            start=True, stop=True,
                        )
                        nc.scalar.activation(
                            out=bcw[:, ch * 512 : (ch + 1) * 512],
                            in_=bc_ps[0:D, :], func=AF.Identity,
                        )
                    nc.vector.tensor_mul(
                        out=AO[(h % 2) * D : (h % 2) * D + D, h // 2, :],
                        in0=O[0:D, :], in1=bcw,
                    )

            # ---- emission schedule --------------------------------------
            # Early slabs on ACT (idle before exps), late slabs on DVE.
            for s in (0, 1):
                _proj_slab(nc, psA, WqT, qT, s, qk_evac_act(QTg, s, bq_sb))
                _proj_slab(nc, psA, WkT, cT, s, qk_evac_act(KTg, s, bk_sb))
            for tb in range(TB):
                v_proj(tb)
            head(0)
            head(1)
            for s in (2, 3):
                _proj_slab(nc, psA, WqT, qT, s, qk_evac_dve(QTg, s, bq_sb))
                _proj_slab(nc, psA, WkT, cT, s, qk_evac_dve(KTg, s, bk_sb))
            head(2)
            head(3)
            for s in (4, 5):
                _proj_slab(nc, psA, WqT, qT, s, qk_evac_dve(QTg, s, bq_sb))
                _proj_slab(nc, psA, WkT, cT, s, qk_evac_dve(KTg, s, bk_sb))
            head(4)
            head(5)
            for s in (6, 7):
                _proj_slab(nc, psA, WqT, qT, s, qk_evac_dve(QTg, s, bq_sb))
                _proj_slab(nc, psA, WkT, cT, s, qk_evac_dve(KTg, s, bk_sb))
            for h in range(6, H):
                head(h)

        # ---- stage 4: out-proj + residual + LayerNorm -------------------
        with (
            tc.tile_pool(name="psY", bufs=1, space="PSUM") as psY,
            tc.tile_pool(name="epi", bufs=1) as epi,
        ):
            for tb in range(TB):
                Y = psY.tile([128, C], F32, name="Y", tag="Y", bufs=2)
                for fp in range(KB // 2):
                    lhsT = AO[:, 2 * fp : 2 * fp + 2, tb * 128 : (tb + 1) * 128]
                    for c0, c1 in ((0, 512), (512, C)):
                        nc.tensor.matmul(
                            Y[:, c0:c1], lhsT,
                            WoT[:, 2 * fp : 2 * fp + 2, c0:c1],
                            start=(fp == 0), stop=(fp == KB // 2 - 1),
                            perf_mode=DR,
                        )
                # residual add with running sum(x); then E[x^2] via a Square
                # activation with accumulator (ACT is idle during the tail)
                x1 = epi.tile([128, C], F32, name="x1", tag="x1", bufs=3)
                sx = epi.tile([128, 1], F32, name="sx", tag="sx", bufs=2)
                nc.vector.scalar_tensor_tensor(
                    out=x1, in0=Y, scalar=0.0, in1=qbo_sb[:, tb, :],
                    op0=ALU.add, op1=ALU.add, accum_out=sx,
                )
                scr = epi.tile([128, C], BF16, name="scr", tag="scr", bufs=2)
                ssq = epi.tile([128, 1], F32, name="ssq", tag="ssq", bufs=2)
                if identity_ln and tb % 2 == 0:
                    nc.vector.scalar_tensor_tensor(
                        out=scr, in0=x1, scalar=0.0, in1=x1,
                        op0=ALU.add, op1=ALU.mult, accum_out=ssq,
                    )
                else:
                    nc.scalar.activation(
                        out=scr, in_=x1, func=AF.Square, accum_out=ssq,
                    )
                mu = epi.tile([128, 1], F32, name="mu", tag="mu", bufs=2)
                nc.vector.tensor_scalar(
                    out=mu, in0=sx, scalar1=1.0 / C, scalar2=None, op0=ALU.mult,
                )
                musq = epi.tile([128, 1], F32, name="musq", tag="musq", bufs=2)
                nc.vector.tensor_scalar(
                    out=musq, in0=mu, scalar1=mu[:, 0:1], scalar2=None,
                    op0=ALU.mult,
                )
                var = epi.tile([128, 1], F32, name="var", tag="var", bufs=2)
                nc.vector.scalar_tensor_tensor(
                    out=var, in0=ssq, scalar=1.0 / C, in1=musq,
                    op0=ALU.mult, op1=ALU.subtract,
                )
                sd = epi.tile([128, 1], F32, name="sd", tag="sd", bufs=2)
                nc.scalar.activation(
                    out=sd, in_=var, func=AF.Sqrt,
                    bias=eps_t[:, 0:1], scale=1.0,
                )
                rs = epi.tile([128, 1], F32, name="rs", tag="rs", bufs=2)
                nc.vector.reciprocal(out=rs, in_=sd)
                nmr = epi.tile([128, 1], F32, name="nmr", tag="nmr", bufs=2)
                nc.vector.tensor_scalar(
                    out=nmr, in0=mu, scalar1=rs[:, 0:1], scalar2=-1.0,
                    op0=ALU.mult, op1=ALU.mult,
                )
                xn = epi.tile([128, C], F32, name="xn", tag="xn", bufs=3)
                if identity_ln and tb % 2 == 1:
                    nc.vector.tensor_scalar(
                        out=xn, in0=x1, scalar1=mu[:, 0:1],
                        scalar2=rs[:, 0:1],
                        op0=ALU.subtract, op1=ALU.mult,
                    )
                else:
                    nc.scalar.activation(
                        out=xn, in_=x1, func=AF.Identity,
                        scale=rs[:, 0:1], bias=nmr[:, 0:1],
                    )
                if identity_ln:
                    out_sb = xn
                else:
                    xg = epi.tile([128, C], F32, name="xg", tag="xg", bufs=3)
                    nc.gpsimd.tensor_mul(out=xg, in0=xn, in1=gamma_bc)
                    xb = epi.tile([128, C], F32, name="xb", tag="xb", bufs=3)
                    nc.vector.tensor_add(out=xb, in0=xg, in1=beta_bc)
                    out_sb = xb
                nc.sync.dma_start(
                    out=_dram_ap(out_t, tb * 128 * C, [[C, 128], [1, C]]),
                    in_=out_sb,
                )


# ---------------------------------------------------------------------------
# Entry point
# ---------------------------------------------------------------------------
_nc_cache = {}


def _get_nc(identity_ln: bool = False):
    if identity_ln not in _nc_cache:
        _install_compile_hook()
        _nc_cache[identity_ln] = build_nc(identity_ln)
    return _nc_cache[identity_ln]


def make_in_maps(inputs: dict) -> list:
    """Host-side marshaling: shard over batch, transpose to feature-major,
    cast matmul operands to fp8 (e4m3), permute Wq/Wk output features."""
    arrs = {k: np.asarray(v, dtype=np.float32) for k, v in inputs.items()}
    def _permw(W):
        Wp = np.zeros((KB8 * 128, C), dtype=np.float32)
        ok = _PERM >= 0
        Wp[ok] = W[_PERM[ok]]
        return np.ascontiguousarray(Wp.T.astype(FP8_NP))

    def _permb(b):
        bp = np.zeros(KB8 * 128, dtype=np.float32)
        ok = _PERM >= 0
        bp[ok] = b[_PERM[ok]]
        return bp

    shared = {
        "WqTp": _permw(arrs["Wq"]),
        "WkTp": _permw(arrs["Wk"]),
        "WvT": np.ascontiguousarray(arrs["Wv"].T.astype(FP8_NP)),
        "WoT": np.ascontiguousarray(arrs["Wo"].T.astype(FP8_NP)),
        "bqp": _permb(arrs["bq"]),
        "bkp": _permb(arrs["bk"]),
        "bv": arrs["bv"],
        "ln_gamma": arrs["ln_gamma"], "ln_beta": arrs["ln_beta"],
    }
    in_maps = []
    for b in range(B):
        m = dict(shared)
        m["qT"] = np.ascontiguousarray(arrs["query"][b].T.astype(FP8_NP))
        m["cT"] = np.ascontiguousarray(arrs["context"][b].T.astype(FP8_NP))
        m["qbo"] = np.ascontiguousarray(arrs["query"][b] + arrs["bo"])
        in_maps.append(m)
    return in_maps


def kernel(**inputs) -> np.ndarray:
    from concourse.bass_utils import run_bass_kernel_spmd

    g = np.asarray(inputs["ln_gamma"], dtype=np.float32)
    b = np.asarray(inputs["ln_beta"], dtype=np.float32)
    identity_ln = bool(np.all(g == 1.0) and np.all(b == 0.0))
    nc = _get_nc(identity_ln)
    in_maps = make_in_maps(inputs)
    res = run_bass_kernel_spmd(nc, in_maps, core_ids=list(range(B)))
    return np.stack([r["out"] for r in res.results]).astype(np.float32)
